# revision 8
# baseline (speedup 1.0000x reference)
"""CenterLossLinear TRN2 kernel: 8-way class/units-sharded.

Per core k (shard size S=12500 of U=100000):
  - logits[:, kS:(k+1)S] = embedding @ fc_w[:, kS:(k+1)S] + fc_b[kS:(k+1)S]
    (float32r matmuls, PSUM f32 accumulation over K=512)
  - gather of centers rows for labels owned by the shard (SWDGE dma_gather),
    center-loss partial sums on DVE,
  - duplicate-label updates combined on the PE via a host-built 0/1 matrix
    (combined = M @ a; only each label's first occurrence carries the group
    sum, the rest are zero rows redirected to a dummy row),
  - new_centers shard = DRAM copy of centers shard + dma_scatter_add of the
    combined f32 update rows.
Host does the label routing (the "all-to-all"), shard slicing and output
concat only.
"""
import os
import sys
import types

sys.path.insert(0, "/opt/trn_rl_repo")

import numpy as np

from concourse import bacc, bass, tile
from concourse import mybir

B, D, U = 1024, 512, 100000
NCORES = 8
S = U // NCORES            # 12500 classes / center rows per core
L = 1024                   # max labels routed to one core (= B, always safe)
C = L // 128               # free-dim slots per partition for gathered rows
NT, TN = 25, 500           # n-tiles per core: 25 x 500 = 12500
KT = 4                     # k-tiles: 4 x 128 = 512
MT = 8                     # m-tiles: 8 x 128 = 1024
ALPHA = 0.5

_CACHE = {}


def _install_trace_shim():
    import antenv

    if "antenv.axon_hooks" in sys.modules:
        return
    mod = types.ModuleType("antenv.axon_hooks")
    mod._hook = None
    mod.set_axon_ntff_profile_hook = lambda h: setattr(mod, "_hook", h)
    mod.get_axon_ntff_profile_hook = lambda: mod._hook
    sys.modules["antenv.axon_hooks"] = mod
    antenv.axon_hooks = mod
    from trn_agent_boot.trn_boot import _ntff_profile_via_ctypes

    mod._hook = _ntff_profile_via_ctypes("/opt/axon/libaxon_pjrt.so")
    import concourse.bass_utils as bu

    bu.upload_artifacts = lambda tmpdir: "local://skipped"


def _build():
    if "nc" in _CACHE:
        return _CACHE["nc"]
    f32 = mybir.dt.float32
    f32r = mybir.dt.float32r
    i16 = mybir.dt.int16

    nc = bacc.Bacc()
    embT = nc.dram_tensor("embT", [D, B], f32, kind="ExternalInput")
    fcw = nc.dram_tensor("fcw", [D, S], f32, kind="ExternalInput")
    fcb = nc.dram_tensor("fcb", [1, S], f32, kind="ExternalInput")
    cin = nc.dram_tensor("cin", [S, D], f32, kind="ExternalInput")
    semb = nc.dram_tensor("semb", [128, C, D], f32, kind="ExternalInput")
    gidx = nc.dram_tensor("gidx", [128, L // 16], i16, kind="ExternalInput")
    sidx = nc.dram_tensor("sidx", [128, L // 16], i16, kind="ExternalInput")
    wmask = nc.dram_tensor("wmask", [128, C], f32, kind="ExternalInput")
    mt = nc.dram_tensor("mt", [C, 128, C, 128], f32, kind="ExternalInput")
    logits = nc.dram_tensor("logits", [B, S], f32, kind="ExternalOutput")
    cout = nc.dram_tensor("cout", [S + 1, D], f32, kind="ExternalOutput")
    lpart = nc.dram_tensor("lpart", [128, 1], f32, kind="ExternalOutput")

    with tile.TileContext(nc) as tc:
        with (
            tc.tile_pool(name="const", bufs=1) as const,
            tc.tile_pool(name="gs", bufs=1) as gs,
            tc.tile_pool(name="stage", bufs=3) as stage,
            tc.tile_pool(name="psum", bufs=4, space="PSUM") as psum,
            tc.tile_pool(name="psumb", bufs=2, space="PSUM") as psumb,
            tc.tile_pool(name="stream2", bufs=2) as stream2,
        ):
            # ---- new_centers: DRAM->DRAM copy of untouched rows (ACT ring) --
            cs = S // 4
            for ch in range(4):
                nc.scalar.dma_start(
                    out=cout[ch * cs : (ch + 1) * cs, :],
                    in_=cin[ch * cs : (ch + 1) * cs, :],
                )

            # ---- gather / loss -------------------------------------------
            gidx_t = gs.tile([128, L // 16], i16)
            sidx_t = gs.tile([128, L // 16], i16)
            w_t = gs.tile([128, C], f32)
            nc.scalar.dma_start(out=gidx_t[:], in_=gidx[:])
            nc.scalar.dma_start(out=sidx_t[:], in_=sidx[:])
            nc.scalar.dma_start(out=w_t[:], in_=wmask[:])

            g_t = gs.tile([128, C, D], f32)
            nc.gpsimd.dma_gather(
                out_ap=g_t[:],
                in_ap=cin[:],
                idxs_ap=gidx_t[:],
                num_idxs=L,
                num_idxs_reg=L,
                elem_size=D,
            )

            # a = w * (semb - g), w holds 0.5 for valid rows / 0 for padding;
            # a_r is the float32r copy the PE consumes.
            a_r = gs.tile([128, C, D], f32r)
            lp_t = gs.tile([128, C], f32)
            lpart_t = gs.tile([128, 1], f32)
            for c in range(C):
                se_c = stream2.tile([128, D], f32, tag="sembc")
                nc.scalar.dma_start(out=se_c[:], in_=semb[:, c, :])
                t_c = stage.tile([128, D], f32, tag="tsub")
                sq_c = stage.tile([128, D], f32, tag="tsq")
                nc.vector.tensor_tensor(
                    out=t_c[:], in0=se_c[:], in1=g_t[:, c, :],
                    op=mybir.AluOpType.subtract,
                )
                nc.vector.tensor_scalar_mul(a_r[:, c, :], t_c[:], w_t[:, c : c + 1])
                nc.vector.tensor_tensor(
                    out=sq_c[:], in0=a_r[:, c, :], in1=a_r[:, c, :],
                    op=mybir.AluOpType.mult,
                )
                nc.vector.tensor_reduce(
                    out=lp_t[:, c : c + 1], in_=sq_c[:],
                    axis=mybir.AxisListType.X, op=mybir.AluOpType.add,
                )
            nc.vector.tensor_reduce(
                out=lpart_t[:], in_=lp_t[:], axis=mybir.AxisListType.X,
                op=mybir.AluOpType.add,
            )
            nc.scalar.dma_start(out=lpart[:], in_=lpart_t[:])

            # ---- combine duplicate updates: comb = M @ a (PE) -------------
            comb_t = gs.tile([128, C, D], f32)
            for ci in range(C):
                mt_f = stream2.tile([128, C, 128], f32, tag="mtraw")
                mt_r = stream2.tile([128, C, 128], f32r, tag="mtrnd")
                nc.scalar.dma_start(out=mt_f[:], in_=mt[ci])
                nc.vector.tensor_copy(mt_r[:], mt_f[:])
                pc = psumb.tile([128, D], mybir.dt.float32, tag="pcomb")
                for cj in range(C):
                    nc.tensor.matmul(
                        pc[:],
                        mt_r[:, cj, :],
                        a_r[:, cj, :],
                        start=(cj == 0),
                        stop=(cj == C - 1),
                    )
                nc.vector.tensor_copy(comb_t[:, ci, :], pc[:])

            # scatter-add the combined updates onto the copied shard (after
            # the copy; Tile orders the WAW). Non-carrier/padding rows are
            # zero and target the dummy row S.
            nc.gpsimd.dma_scatter_add(
                out_ap=cout[:],
                in_ap=comb_t[:],
                idxs_ap=sidx_t[:],
                num_idxs=L,
                num_idxs_reg=L,
                elem_size=D,
            )

            # ---- bias broadcast [128, S] via K=1 ones-matmul ---------------
            ones_f = const.tile([1, 128], f32)
            ones_r = const.tile([1, 128], f32r)
            nc.vector.memset(ones_f[:], 1.0)
            nc.vector.tensor_copy(ones_r[:], ones_f[:])
            bias_rep = const.tile([128, S], f32)
            for n in range(NT):
                fb_f = stream2.tile([1, TN], f32, tag="fbraw")
                fb_r = stream2.tile([1, TN], f32r, tag="fbrnd")
                nc.scalar.dma_start(out=fb_f[:], in_=fcb[:, n * TN : (n + 1) * TN])
                nc.vector.tensor_copy(fb_r[:], fb_f[:])
                pb = psumb.tile([128, TN], f32, tag="pbias")
                nc.tensor.matmul(pb[:], ones_r[:], fb_r[:], start=True, stop=True)
                nc.scalar.copy(bias_rep[:, n * TN : (n + 1) * TN], pb[:])

            # ---- embT resident in SBUF as float32r -------------------------
            etr = []
            for k in range(KT):
                ef = stream2.tile([128, B], f32, tag="eraw")
                nc.sync.dma_start(out=ef[:], in_=embT[k * 128 : (k + 1) * 128, :])
                er = const.tile([128, B], f32r, tag=f"etr{k}")
                nc.vector.tensor_copy(er[:], ef[:])
                etr.append(er)

            # ---- main matmul loop ------------------------------------------
            with (
                tc.tile_pool(name="rfp", bufs=3) as rfp,
                tc.tile_pool(name="rrp", bufs=2) as rrp,
            ):
                for n in range(NT):
                    rr_all = rrp.tile([128, KT, TN], f32r, tag="rrnd")
                    for k in range(KT):
                        rf = rfp.tile([128, TN], f32, tag="rraw")
                        nc.sync.dma_start(
                            out=rf[:],
                            in_=fcw[k * 128 : (k + 1) * 128, n * TN : (n + 1) * TN],
                        )
                        nc.vector.tensor_copy(rr_all[:, k, :], rf[:])
                    for m in range(MT):
                        pt = psum.tile([128, TN], f32)
                        for k in range(KT):
                            nc.tensor.matmul(
                                pt[:],
                                etr[k][:, m * 128 : (m + 1) * 128],
                                rr_all[:, k, :],
                                start=(k == 0),
                                stop=(k == KT - 1),
                            )
                        st = stage.tile([128, TN], f32, tag="lst")
                        nc.vector.tensor_tensor(
                            out=st[:], in0=pt[:],
                            in1=bias_rep[:, n * TN : (n + 1) * TN],
                            op=mybir.AluOpType.add,
                        )
                        nc.sync.dma_start(
                            out=logits[m * 128 : (m + 1) * 128, n * TN : (n + 1) * TN],
                            in_=st[:],
                        )

    nc.compile()
    _CACHE["nc"] = nc
    return nc


def _shard_inputs(embedding, labels, centers, fc_w, fc_b):
    embedding = np.ascontiguousarray(embedding, dtype=np.float32)
    labels = np.asarray(labels).astype(np.int64)
    centers = np.ascontiguousarray(centers, dtype=np.float32)
    fc_w = np.ascontiguousarray(fc_w, dtype=np.float32)
    fc_b = np.ascontiguousarray(fc_b, dtype=np.float32).reshape(1, U)

    embT = np.ascontiguousarray(embedding.T)
    in_maps = []
    for k in range(NCORES):
        sel = np.nonzero((labels >= k * S) & (labels < (k + 1) * S))[0]
        cnt = len(sel)
        assert cnt <= L
        loc = (labels[sel] - k * S).astype(np.int16)

        gi = np.zeros(L, dtype=np.int16)
        si = np.full(L, S, dtype=np.int16)  # padding targets the dummy row
        wm = np.zeros(L, dtype=np.float32)
        se = np.zeros((L, D), dtype=np.float32)
        gi[:cnt] = loc
        wm[:cnt] = ALPHA
        se[:cnt] = embedding[sel]

        # duplicate combining: only the first occurrence of each label value
        # scatters (with the group-summed update); the rest go to row S.
        first = np.zeros(cnt, dtype=bool)
        first[np.unique(loc, return_index=True)[1]] = True
        si[:cnt] = np.where(first, loc, S)
        M = np.zeros((L, L), dtype=np.float32)
        eq = (loc[:, None] == loc[None, :]).astype(np.float32)
        M[:cnt, :cnt] = eq * first[:, None]
        mt5 = np.ascontiguousarray(
            M.T.reshape(C, 128, C, 128).transpose(2, 1, 0, 3)
        )

        # token i lives at partition i%128, slot i//128; idx stream is wrapped
        # in 16 partitions and replicated to all 128.
        semb = np.ascontiguousarray(se.reshape(C, 128, D).transpose(1, 0, 2))
        wmask = np.ascontiguousarray(wm.reshape(C, 128).T)
        gidx = np.tile(gi.reshape(L // 16, 16).T, (8, 1))
        sidx = np.tile(si.reshape(L // 16, 16).T, (8, 1))

        in_maps.append(
            {
                "embT": embT,
                "fcw": np.ascontiguousarray(fc_w[:, k * S : (k + 1) * S]),
                "fcb": np.ascontiguousarray(fc_b[:, k * S : (k + 1) * S]),
                "cin": np.ascontiguousarray(centers[k * S : (k + 1) * S]),
                "semb": semb,
                "gidx": np.ascontiguousarray(gidx),
                "sidx": np.ascontiguousarray(sidx),
                "wmask": wmask,
                "mt": mt5,
            }
        )
    return in_maps


def kernel(embedding, labels, centers, fc_w, fc_b):
    trace = os.environ.get("KERNEL_TRACE") == "1"
    if trace:
        _install_trace_shim()
    from concourse.bass_utils import run_bass_kernel_spmd

    nc = _build()
    in_maps = _shard_inputs(embedding, labels, centers, fc_w, fc_b)
    res = run_bass_kernel_spmd(nc, in_maps, list(range(NCORES)), trace=trace)
    if trace:
        _CACHE["exec_time_ns"] = res.exec_time_ns

    logits = np.concatenate([res.results[k]["logits"] for k in range(NCORES)], axis=1)
    new_centers = np.concatenate(
        [res.results[k]["cout"][:S] for k in range(NCORES)], axis=0
    )
    lsum = sum(float(res.results[k]["lpart"].sum()) for k in range(NCORES))
    center_loss = np.float32(4.0 * lsum / (B * D))
    return logits, center_loss, new_centers


# revision 12
# speedup vs baseline: 1.0105x; 1.0105x over previous
"""CenterLossLinear TRN2 kernel: 8-way class/units-sharded.

Per core k (shard size S=12500 of U=100000):
  - logits[:, kS:(k+1)S] = embedding @ fc_w[:, kS:(k+1)S] + fc_b[kS:(k+1)S]
    (float32r matmuls, PSUM f32 accumulation over K=512)
  - gather of centers rows for labels owned by the shard (SWDGE dma_gather),
    center-loss partial sums on DVE,
  - duplicate-label updates combined on the PE via a host-built 0/1 matrix
    (combined = M @ a; only each label's first occurrence carries the group
    sum, the rest are zero rows redirected to a dummy row),
  - new_centers shard = DRAM copy of centers shard + dma_scatter_add of the
    combined f32 update rows.
Host does the label routing (the "all-to-all"), shard slicing and output
concat only.
"""
import os
import sys
import types

sys.path.insert(0, "/opt/trn_rl_repo")

import numpy as np

from concourse import bacc, bass, tile
from concourse import mybir

B, D, U = 1024, 512, 100000
NCORES = 8
S = U // NCORES            # 12500 classes / center rows per core
L = 1024                   # max labels routed to one core (= B, always safe)
C = L // 128               # free-dim slots per partition for gathered rows
NT, TN = 25, 500           # n-tiles per core: 25 x 500 = 12500
KT = 4                     # k-tiles: 4 x 128 = 512
MT = 8                     # m-tiles: 8 x 128 = 1024
ALPHA = 0.5

_CACHE = {}


def _install_trace_shim():
    import antenv

    if "antenv.axon_hooks" in sys.modules:
        return
    mod = types.ModuleType("antenv.axon_hooks")
    mod._hook = None
    mod.set_axon_ntff_profile_hook = lambda h: setattr(mod, "_hook", h)
    mod.get_axon_ntff_profile_hook = lambda: mod._hook
    sys.modules["antenv.axon_hooks"] = mod
    antenv.axon_hooks = mod
    from trn_agent_boot.trn_boot import _ntff_profile_via_ctypes

    mod._hook = _ntff_profile_via_ctypes("/opt/axon/libaxon_pjrt.so")
    import concourse.bass_utils as bu

    bu.upload_artifacts = lambda tmpdir: "local://skipped"


def _build():
    if "nc" in _CACHE:
        return _CACHE["nc"]
    f32 = mybir.dt.float32
    f32r = mybir.dt.float32r
    i16 = mybir.dt.int16

    nc = bacc.Bacc()
    embT = nc.dram_tensor("embT", [D, B], f32, kind="ExternalInput")
    fcw = nc.dram_tensor("fcw", [D, S], f32, kind="ExternalInput")
    fcb = nc.dram_tensor("fcb", [1, S], f32, kind="ExternalInput")
    cin = nc.dram_tensor("cin", [S, D], f32, kind="ExternalInput")
    semb = nc.dram_tensor("semb", [128, C, D], f32, kind="ExternalInput")
    gidx = nc.dram_tensor("gidx", [128, L // 16], i16, kind="ExternalInput")
    sidx = nc.dram_tensor("sidx", [128, L // 16], i16, kind="ExternalInput")
    wmask = nc.dram_tensor("wmask", [128, C], f32, kind="ExternalInput")
    mt = nc.dram_tensor("mt", [C, 128, C, 128], f32, kind="ExternalInput")
    logits = nc.dram_tensor("logits", [B, S], f32, kind="ExternalOutput")
    cout = nc.dram_tensor("cout", [S + 1, D], f32, kind="ExternalOutput")
    lpart = nc.dram_tensor("lpart", [128, 1], f32, kind="ExternalOutput")

    with tile.TileContext(nc) as tc:
        with (
            tc.tile_pool(name="const", bufs=1) as const,
            tc.tile_pool(name="gs", bufs=1) as gs,
            tc.tile_pool(name="stage", bufs=3) as stage,
            tc.tile_pool(name="psum", bufs=4, space="PSUM") as psum,
            tc.tile_pool(name="psumb", bufs=2, space="PSUM") as psumb,
            tc.tile_pool(name="stream2", bufs=2) as stream2,
        ):
            # ---- gather / loss -------------------------------------------
            gidx_t = gs.tile([128, L // 16], i16)
            sidx_t = gs.tile([128, L // 16], i16)
            w_t = gs.tile([128, C], f32)
            nc.scalar.dma_start(out=gidx_t[:], in_=gidx[:])
            nc.scalar.dma_start(out=sidx_t[:], in_=sidx[:])
            nc.scalar.dma_start(out=w_t[:], in_=wmask[:])

            g_t = gs.tile([128, C, D], f32)
            nc.gpsimd.dma_gather(
                out_ap=g_t[:],
                in_ap=cin[:],
                idxs_ap=gidx_t[:],
                num_idxs=L,
                num_idxs_reg=L,
                elem_size=D,
            )

            # a = w * (semb - g), w holds 0.5 for valid rows / 0 for padding;
            # a_r is the float32r copy the PE consumes.
            a_r = gs.tile([128, C, D], f32r)
            lp_t = gs.tile([128, C], f32)
            lpart_t = gs.tile([128, 1], f32)
            for c in range(C):
                se_c = stream2.tile([128, D], f32, tag="sembc")
                nc.scalar.dma_start(out=se_c[:], in_=semb[:, c, :])
                t_c = stage.tile([128, D], f32, tag="tsub")
                sq_c = stage.tile([128, D], f32, tag="tsq")
                nc.vector.tensor_tensor(
                    out=t_c[:], in0=se_c[:], in1=g_t[:, c, :],
                    op=mybir.AluOpType.subtract,
                )
                nc.vector.tensor_scalar_mul(a_r[:, c, :], t_c[:], w_t[:, c : c + 1])
                nc.vector.tensor_tensor(
                    out=sq_c[:], in0=a_r[:, c, :], in1=a_r[:, c, :],
                    op=mybir.AluOpType.mult,
                )
                nc.vector.tensor_reduce(
                    out=lp_t[:, c : c + 1], in_=sq_c[:],
                    axis=mybir.AxisListType.X, op=mybir.AluOpType.add,
                )
            nc.vector.tensor_reduce(
                out=lpart_t[:], in_=lp_t[:], axis=mybir.AxisListType.X,
                op=mybir.AluOpType.add,
            )
            nc.scalar.dma_start(out=lpart[:], in_=lpart_t[:])

            # ---- combine duplicate updates: comb = M @ a (PE) -------------
            comb_t = gs.tile([128, C, D], f32)
            for ci in range(C):
                mt_f = stream2.tile([128, C, 128], f32, tag="mtraw")
                mt_r = stream2.tile([128, C, 128], f32r, tag="mtrnd")
                nc.scalar.dma_start(out=mt_f[:], in_=mt[ci])
                nc.vector.tensor_copy(mt_r[:], mt_f[:])
                pc = psumb.tile([128, D], mybir.dt.float32, tag="pcomb")
                for cj in range(C):
                    nc.tensor.matmul(
                        pc[:],
                        mt_r[:, cj, :],
                        a_r[:, cj, :],
                        start=(cj == 0),
                        stop=(cj == C - 1),
                    )
                nc.vector.tensor_copy(comb_t[:, ci, :], pc[:])

            # ---- bias broadcast [128, S] via K=1 ones-matmul ---------------
            ones_f = const.tile([1, 128], f32)
            ones_r = const.tile([1, 128], f32r)
            nc.vector.memset(ones_f[:], 1.0)
            nc.vector.tensor_copy(ones_r[:], ones_f[:])
            bias_rep = const.tile([128, S], f32)
            for n in range(NT):
                fb_f = stream2.tile([1, TN], f32, tag="fbraw")
                fb_r = stream2.tile([1, TN], f32r, tag="fbrnd")
                nc.scalar.dma_start(out=fb_f[:], in_=fcb[:, n * TN : (n + 1) * TN])
                nc.vector.tensor_copy(fb_r[:], fb_f[:])
                pb = psumb.tile([128, TN], f32, tag="pbias")
                nc.tensor.matmul(pb[:], ones_r[:], fb_r[:], start=True, stop=True)
                nc.scalar.copy(bias_rep[:, n * TN : (n + 1) * TN], pb[:])

            # ---- embT resident in SBUF as float32r -------------------------
            etr = []
            for k in range(KT):
                ef = stream2.tile([128, B], f32, tag="eraw")
                nc.sync.dma_start(out=ef[:], in_=embT[k * 128 : (k + 1) * 128, :])
                er = const.tile([128, B], f32r, tag=f"etr{k}")
                nc.vector.tensor_copy(er[:], ef[:])
                etr.append(er)

            # ---- main matmul loop ------------------------------------------
            # SP ring: fcw loads only (latency-critical, prefetched one n
            # ahead). ACT ring: logits stores + one centers-copy chunk per n
            # (stores are fire-and-forget; the copy fills ring slack).
            cchunk = S // NT
            with (
                tc.tile_pool(name="rfp", bufs=5) as rfp,
                tc.tile_pool(name="rrp", bufs=2) as rrp,
            ):
                def issue_loads(n):
                    rfs = []
                    for k in range(KT):
                        rf = rfp.tile([128, TN], f32, tag="rraw", name=f"rf{n}_{k}")
                        nc.sync.dma_start(
                            out=rf[:],
                            in_=fcw[k * 128 : (k + 1) * 128, n * TN : (n + 1) * TN],
                        )
                        rfs.append(rf)
                    return rfs

                def cast_loads(n, rfs):
                    rr_all = rrp.tile([128, KT, TN], f32r, tag="rrnd",
                                      name=f"rr{n}")
                    for k in range(KT):
                        nc.vector.tensor_copy(rr_all[:, k, :], rfs[k][:])
                    return rr_all

                pending = issue_loads(0)
                rr_cur = cast_loads(0, pending)
                for n in range(NT):
                    if n + 1 < NT:
                        pending = issue_loads(n + 1)
                    for m in range(MT):
                        pt = psum.tile([128, TN], f32)
                        for k in range(KT):
                            nc.tensor.matmul(
                                pt[:],
                                etr[k][:, m * 128 : (m + 1) * 128],
                                rr_cur[:, k, :],
                                start=(k == 0),
                                stop=(k == KT - 1),
                            )
                        st = stage.tile([128, TN], f32, tag="lst")
                        nc.vector.tensor_tensor(
                            out=st[:], in0=pt[:],
                            in1=bias_rep[:, n * TN : (n + 1) * TN],
                            op=mybir.AluOpType.add,
                        )
                        nc.scalar.dma_start(
                            out=logits[m * 128 : (m + 1) * 128, n * TN : (n + 1) * TN],
                            in_=st[:],
                        )
                    nc.scalar.dma_start(
                        out=cout[n * cchunk : (n + 1) * cchunk, :],
                        in_=cin[n * cchunk : (n + 1) * cchunk, :],
                    )
                    if n + 1 < NT:
                        rr_cur = cast_loads(n + 1, pending)

            # scatter-add the combined updates onto the copied shard — emitted
            # last so Tile orders it after every copy chunk (WAW on cout).
            # Non-carrier/padding rows are zero and target the dummy row S.
            nc.gpsimd.dma_scatter_add(
                out_ap=cout[:],
                in_ap=comb_t[:],
                idxs_ap=sidx_t[:],
                num_idxs=L,
                num_idxs_reg=L,
                elem_size=D,
            )

    nc.compile()
    _CACHE["nc"] = nc
    return nc


def _shard_inputs(embedding, labels, centers, fc_w, fc_b):
    embedding = np.ascontiguousarray(embedding, dtype=np.float32)
    labels = np.asarray(labels).astype(np.int64)
    centers = np.ascontiguousarray(centers, dtype=np.float32)
    fc_w = np.ascontiguousarray(fc_w, dtype=np.float32)
    fc_b = np.ascontiguousarray(fc_b, dtype=np.float32).reshape(1, U)

    embT = np.ascontiguousarray(embedding.T)
    in_maps = []
    for k in range(NCORES):
        sel = np.nonzero((labels >= k * S) & (labels < (k + 1) * S))[0]
        cnt = len(sel)
        assert cnt <= L
        loc = (labels[sel] - k * S).astype(np.int16)

        gi = np.zeros(L, dtype=np.int16)
        si = np.full(L, S, dtype=np.int16)  # padding targets the dummy row
        wm = np.zeros(L, dtype=np.float32)
        se = np.zeros((L, D), dtype=np.float32)
        gi[:cnt] = loc
        wm[:cnt] = ALPHA
        se[:cnt] = embedding[sel]

        # duplicate combining: only the first occurrence of each label value
        # scatters (with the group-summed update); the rest go to row S.
        first = np.zeros(cnt, dtype=bool)
        first[np.unique(loc, return_index=True)[1]] = True
        si[:cnt] = np.where(first, loc, S)
        M = np.zeros((L, L), dtype=np.float32)
        eq = (loc[:, None] == loc[None, :]).astype(np.float32)
        M[:cnt, :cnt] = eq * first[:, None]
        mt5 = np.ascontiguousarray(
            M.T.reshape(C, 128, C, 128).transpose(2, 1, 0, 3)
        )

        # token i lives at partition i%128, slot i//128; idx stream is wrapped
        # in 16 partitions and replicated to all 128.
        semb = np.ascontiguousarray(se.reshape(C, 128, D).transpose(1, 0, 2))
        wmask = np.ascontiguousarray(wm.reshape(C, 128).T)
        gidx = np.tile(gi.reshape(L // 16, 16).T, (8, 1))
        sidx = np.tile(si.reshape(L // 16, 16).T, (8, 1))

        in_maps.append(
            {
                "embT": embT,
                "fcw": np.ascontiguousarray(fc_w[:, k * S : (k + 1) * S]),
                "fcb": np.ascontiguousarray(fc_b[:, k * S : (k + 1) * S]),
                "cin": np.ascontiguousarray(centers[k * S : (k + 1) * S]),
                "semb": semb,
                "gidx": np.ascontiguousarray(gidx),
                "sidx": np.ascontiguousarray(sidx),
                "wmask": wmask,
                "mt": mt5,
            }
        )
    return in_maps


def kernel(embedding, labels, centers, fc_w, fc_b):
    trace = os.environ.get("KERNEL_TRACE") == "1"
    if trace:
        _install_trace_shim()
    from concourse.bass_utils import run_bass_kernel_spmd

    nc = _build()
    in_maps = _shard_inputs(embedding, labels, centers, fc_w, fc_b)
    res = run_bass_kernel_spmd(nc, in_maps, list(range(NCORES)), trace=trace)
    if trace:
        _CACHE["exec_time_ns"] = res.exec_time_ns

    logits = np.concatenate([res.results[k]["logits"] for k in range(NCORES)], axis=1)
    new_centers = np.concatenate(
        [res.results[k]["cout"][:S] for k in range(NCORES)], axis=0
    )
    lsum = sum(float(res.results[k]["lpart"].sum()) for k in range(NCORES))
    center_loss = np.float32(4.0 * lsum / (B * D))
    return logits, center_loss, new_centers
